# revision 1
# baseline (speedup 1.0000x reference)
"""Trainium2 Bass kernel for nn_CausalSelfAttention_22703197127379.

Reference computation (k/v are dead code — attention is stubbed to RoPE(q)):
    q    = hidden @ w_qkv[:, :4096]           # [8192, 4096]
    qr   = rope_neox(q, positions)            # per-head rotate-half RoPE
    out  = qr @ w_o                           # [8192, 4096]

Distribution: data-parallel over tokens — core c owns rows c*1024..(c+1)*1024.
No collectives; host concatenates the 8 shards.

Per-core device kernel (all matmuls f32r = full-rate fp32 mode on TensorE):
  phase 1: Q^T[h] = sum_e wq[e,h].T @ xT[e,t] accumulated in PSUM, then RoPE
           applied as qs = Q^T*C + swap_halves(Q^T*S) (swap = partition-swap
           DMA, signs baked into the host-built S table), bounced to DRAM.
  phase 2: outT[f, t] = sum_h wo[h,f].T @ qT[h,t], PSUM-accumulated over all
           32 head blocks, written transposed; host transposes back.
"""

import sys

if "/opt/trn_rl_repo" not in sys.path:
    sys.path.insert(0, "/opt/trn_rl_repo")

import numpy as np

NCORES = 8
T, E, QS = 8192, 4096, 4096
TL = T // NCORES          # 1024 tokens per core
NH = 32                   # q heads
HD = 128                  # head dim
HALF = HD // 2
EB = E // 128             # 32 contraction blocks
QB = QS // 128            # 32 head blocks
ROPE_THETA = 10000.0

_NC_CACHE = {}

# tuning knobs (read at build time)
TUNE = {
    "ps1_bufs": 4,
    "rope_bufs": 2,
    "wqp_bufs": 2,
    "wop_bufs": 8,
    "ost_bufs": 4,
    "skip_reload": False,   # timing experiment only — wrong results
    "skip_rope": False,     # timing experiment only — wrong results
}


def _build_nc(loop_iters=None):
    """Build the per-core NEFF. loop_iters wraps the whole compute body in a
    hardware For_i loop (timing-only builds; data goes stale after iter 0)."""
    import contextlib

    import concourse.bacc as bacc
    import concourse.mybir as mybir
    from concourse.tile import TileContext

    F32 = mybir.dt.float32
    F32R = mybir.dt.float32r

    nc = bacc.Bacc()
    # all inputs arrive pre-rearranged on host so every DMA is contiguous:
    # xT[p, eb*TL + t]            = hidden_shard.T[eb*128 + p, t]
    # wq[h*128 + p, eb*HD + f]    = w_q[eb*128 + p, h*HD + f]
    # wo[(fq*QB + h)*128 + p, f]  = w_o[h*128 + p, fq*512 + f]
    xT = nc.declare_dram_parameter("xT", [128, EB * TL], F32R, isOutput=False)
    wq = nc.declare_dram_parameter("wq", [NH * 128, EB * HD], F32R, isOutput=False)
    wo = nc.declare_dram_parameter("wo", [(E // 512) * QB * 128, 512], F32R,
                                   isOutput=False)
    Ct = nc.declare_dram_parameter("Ct", [HD, TL], F32, isOutput=False)
    St = nc.declare_dram_parameter("St", [HD, TL], F32, isOutput=False)
    outT = nc.declare_dram_parameter("outT", [E, TL], F32, isOutput=True)
    qTd = nc.dram_tensor("qTd", [QS, TL], F32R)

    with TileContext(nc) as tc:
        loop_cm = (tc.For_i(0, loop_iters, 1) if loop_iters
                   else contextlib.nullcontext())
        with loop_cm:
            _emit_body(nc, tc, mybir, xT, wq, wo, Ct, St, outT, qTd)

    nc.finalize()
    return nc


def _emit_body(nc, tc, mybir, xT, wq, wo, Ct, St, outT, qTd):
    F32 = mybir.dt.float32
    F32R = mybir.dt.float32r
    if True:
        with tc.tile_pool(name="big", bufs=1) as big:
            # xT resident, e-block-major: X[:, eb*TL + t] = xT[eb*128 + p, t]
            X = big.tile([128, EB * TL], F32R)
            nc.sync.dma_start(out=X[:], in_=xT[:])

            # ---------------- phase 1: Q^T per head + RoPE + bounce ----------
            with tc.tile_pool(name="wqp", bufs=TUNE["wqp_bufs"]) as wqp, \
                 tc.tile_pool(name="tab", bufs=1) as tab, \
                 tc.tile_pool(name="rope", bufs=TUNE["rope_bufs"]) as rope, \
                 tc.tile_pool(name="ps1", bufs=TUNE["ps1_bufs"], space="PSUM") as ps1:
                ct = tab.tile([HD, TL], F32, tag="ct")
                nc.sync.dma_start(out=ct[:], in_=Ct[:])
                stt = tab.tile([HD, TL], F32, tag="st")
                nc.sync.dma_start(out=stt[:], in_=St[:])

                for h in range(NH):
                    wqh = wqp.tile([128, EB * HD], F32R, tag="wqh")
                    nc.sync.dma_start(out=wqh[:],
                                      in_=wq[h * 128:(h + 1) * 128, :])
                    u = rope.tile([128, TL], F32, tag="u")
                    qs = rope.tile([128, TL], F32, tag="qs")
                    v = rope.tile([128, TL], F32, tag="v")
                    qr = rope.tile([128, TL], F32R, tag="qr")
                    pss1 = [ps1.tile([128, 512], F32, tag="ps1", name=f"ps1_{h}_{i}")
                            for i in range(TL // 512)]
                    for tch in range(TL // 512):
                        for eb in range(EB):
                            nc.tensor.matmul(
                                pss1[tch][:],
                                wqh[:, eb * HD:(eb + 1) * HD],
                                X[:, eb * TL + tch * 512: eb * TL + tch * 512 + 512],
                                start=(eb == 0), stop=(eb == EB - 1),
                            )
                    for tch in range(TL // 512):
                        ps = pss1[tch]
                        sl = slice(tch * 512, tch * 512 + 512)
                        if TUNE["skip_rope"]:
                            nc.vector.tensor_copy(qr[:, sl], ps[:])
                            continue
                        nc.vector.tensor_mul(u[:, sl], ps[:], stt[:, sl])
                        nc.vector.tensor_mul(qs[:, sl], ps[:], ct[:, sl])
                    if not TUNE["skip_rope"]:
                        # rotate-half: v = swap_halves(u) via partition-offset DMA
                        nc.sync.dma_start(out=v[0:HALF, :], in_=u[HALF:HD, :])
                        nc.sync.dma_start(out=v[HALF:HD, :], in_=u[0:HALF, :])
                        nc.vector.tensor_add(qr[:], qs[:], v[:])
                    nc.sync.dma_start(out=qTd[h * HD:(h + 1) * HD, :], in_=qr[:])

        # ---------------- phase 2: outT = sum_h wo[h].T @ qT[h] --------------
        # per-head qT tiles (reusing X's freed space) so each head's matmuls
        # only wait on that head's reload DMA.
        with tc.tile_pool(name="qts", bufs=1) as qts, \
             tc.tile_pool(name="wop", bufs=TUNE["wop_bufs"]) as wop, \
             tc.tile_pool(name="ost", bufs=TUNE["ost_bufs"]) as ost, \
             tc.tile_pool(name="ps2", bufs=8, space="PSUM") as ps2:
            qt = []
            for h in range(NH):
                qth = qts.tile([128, TL], F32R, tag=f"qt{h}", name=f"qt_{h}")
                if not TUNE["skip_reload"]:
                    nc.sync.dma_start(out=qth[:], in_=qTd[h * HD:(h + 1) * HD, :])
                qt.append(qth)

            for fq in range(E // 512):
                pss = [ps2.tile([128, 512], F32, tag="ps2", name=f"pss_{fq}_{i}")
                       for i in range(8)]
                for h in range(QB):
                    woh = wop.tile([128, 512], F32R, tag="woh")
                    r0 = (fq * QB + h) * 128
                    nc.sync.dma_start(out=woh[:], in_=wo[r0:r0 + 128, :])
                    for fb in range(4):
                        for t2 in range(2):
                            nc.tensor.matmul(
                                pss[fb * 2 + t2][:],
                                woh[:, fb * 128:(fb + 1) * 128],
                                qt[h][:, t2 * 512: t2 * 512 + 512],
                                start=(h == 0), stop=(h == QB - 1),
                            )
                for fb in range(4):
                    for t2 in range(2):
                        o = ost.tile([128, 512], F32, tag="ost")
                        nc.vector.tensor_copy(o[:], pss[fb * 2 + t2][:])
                        nc.sync.dma_start(
                            out=outT[fq * 512 + fb * 128: fq * 512 + (fb + 1) * 128,
                                     t2 * 512:(t2 + 1) * 512],
                            in_=o[:])


def _get_nc(loop_iters=None):
    key = ("nc", loop_iters)
    if key not in _NC_CACHE:
        _NC_CACHE[key] = _build_nc(loop_iters)
    return _NC_CACHE[key]


def _rope_tables(positions):
    # mirrors reference fp32 math: inv_freq f32, freqs f32, cos/sin f32
    half = np.float32(HALF)
    inv_freq = (1.0 / (ROPE_THETA ** (np.arange(HALF, dtype=np.float32) / half))
                ).astype(np.float32)
    freqs = positions.astype(np.float32)[:, None] * inv_freq[None, :]  # [T, 64]
    cos = np.cos(freqs).astype(np.float32)
    sin = np.sin(freqs).astype(np.float32)
    # qT layout tables: Ct[d, t] = cos[t, d%64]
    # St[d, t] = +sin[t, d] for d<64, -sin[t, d-64] for d>=64, so that
    # qs + swap_halves(q * St) == neox rope of q.
    Ct = np.concatenate([cos.T, cos.T], axis=0)    # [128, T]
    St = np.concatenate([sin.T, -sin.T], axis=0)   # [128, T]
    return np.ascontiguousarray(Ct), np.ascontiguousarray(St)


def build_in_maps(hidden_states, positions, w_qkv, w_o):
    hidden = np.asarray(hidden_states, dtype=np.float32)
    pos = np.asarray(positions)
    wq_nat = np.asarray(w_qkv, dtype=np.float32)[:, :QS]
    wo_nat = np.asarray(w_o, dtype=np.float32)
    # pre-rearranged layouts (see _build_nc comments)
    wq = np.ascontiguousarray(
        wq_nat.reshape(EB, 128, NH, HD).transpose(2, 1, 0, 3)
        .reshape(NH * 128, EB * HD))
    wo = np.ascontiguousarray(
        wo_nat.reshape(QB, 128, E // 512, 512).transpose(2, 0, 1, 3)
        .reshape((E // 512) * QB * 128, 512))
    Ct, St = _rope_tables(pos)
    in_maps = []
    for c in range(NCORES):
        sl = slice(c * TL, (c + 1) * TL)
        xTc = np.ascontiguousarray(
            hidden[sl].T.reshape(EB, 128, TL).transpose(1, 0, 2)
            .reshape(128, EB * TL))
        in_maps.append({
            "xT": xTc,
            "wq": wq,
            "wo": wo,
            "Ct": np.ascontiguousarray(Ct[:, sl]),
            "St": np.ascontiguousarray(St[:, sl]),
        })
    return in_maps


def kernel(hidden_states, positions, w_qkv, w_o):
    from concourse.bass_utils import run_bass_kernel_spmd

    nc = _get_nc()
    in_maps = build_in_maps(hidden_states, positions, w_qkv, w_o)
    res = run_bass_kernel_spmd(nc, in_maps, core_ids=list(range(NCORES)))
    out = np.concatenate(
        [np.asarray(res.results[c]["outT"]).T for c in range(NCORES)], axis=0)
    return np.ascontiguousarray(out.astype(np.float32))



# revision 4
# speedup vs baseline: 15203.0833x; 15203.0833x over previous
"""Trainium2 Bass kernel for nn_CausalSelfAttention_22703197127379.

Reference computation (k/v are dead code — attention is stubbed to RoPE(q)):
    q    = hidden @ w_qkv[:, :4096]           # [8192, 4096]
    qr   = rope_neox(q, positions)            # per-head rotate-half RoPE
    out  = qr @ w_o                           # [8192, 4096]

Distribution: data-parallel over tokens — core c owns rows c*1024..(c+1)*1024.
No collectives; host concatenates the 8 shards.

Design (measured on HW: the TensorE stream saturates at ~1 bf16 row/cycle,
so everything else must hide under the 4096-matmul stream):
  * all matmul operands bf16 (PSUM accumulation stays f32): 3x less DMA +
    SBUF traffic than f32r; rel-err ~3e-3 vs the 2e-2 gate.
  * q stays resident in SBUF between the two matmuls — no qT DRAM bounce,
    no phase-2 reload stall.
  * X loaded as 32 per-e-block DMAs and weight tiles as split DMAs so the
    first matmuls start after ~100KB lands, not after the full 8MB.
  * RoPE reads each PSUM group once (copy to SBUF), math runs from SBUF;
    rotate-half via partition-swap DMA with signs folded into the host-built
    sin table.
  * post-schedule BIR pass drops LDWEIGHTS whose stationary matches the
    previous one (harmless; trims the instruction stream).

Per-core device kernel:
  phase 1: for each head h: Q^T[h] = sum_eb wq[eb,h].T @ x[eb] in PSUM;
           RoPE applied as qt = ps*C + swap_halves(ps*S), cast to bf16 into
           the persistent qt[h] SBUF tile.
  phase 2: outT[f,t] = sum_h wo[h,f].T @ qt[h], PSUM-accumulated over all
           32 head blocks, written transposed; host transposes back.
"""

import sys

if "/opt/trn_rl_repo" not in sys.path:
    sys.path.insert(0, "/opt/trn_rl_repo")

import numpy as np
import ml_dtypes

BF16_NP = ml_dtypes.bfloat16

NCORES = 8
T, E, QS = 8192, 4096, 4096
TL = T // NCORES          # 1024 tokens per core
NH = 32                   # q heads
HD = 128                  # head dim
HALF = HD // 2
EB = E // 128             # 32 contraction blocks
QB = QS // 128            # 32 head blocks
ROPE_THETA = 10000.0

_NC_CACHE = {}

TUNE = {
    "dedupe": True,
    "wq_split": 2,        # DMAs per wq head tile
    "ps1_bufs": 6,
    "wqp_bufs": 3,
    "wop_bufs": 8,
    "rope_bufs": 3,
    "ost_bufs": 4,
}


def _build_nc(loop_iters=None, timing=False):
    """Build the per-core NEFF. loop_iters wraps the compute body in a
    hardware For_i loop (timing-only builds; data goes stale after iter 0).
    timing=True swaps all I/O parameters for internal DRAM tensors (garbage
    contents, nothing shipped over the tunnel) plus a tiny sink output."""
    import contextlib

    import concourse.bacc as bacc
    import concourse.mybir as mybir
    from concourse.tile import TileContext

    F32 = mybir.dt.float32
    BF16 = mybir.dt.bfloat16

    nc = bacc.Bacc()
    # all inputs arrive pre-rearranged on host so every DMA is contiguous:
    # xT[p, eb*TL + t]            = bf16(hidden_shard.T)[eb*128 + p, t]
    # wq[h*128 + p, eb*HD + f]    = bf16(w_q)[eb*128 + p, h*HD + f]
    # wo[(fq*QB + h)*128 + p, f]  = bf16(w_o)[h*128 + p, fq*512 + f]
    if timing:
        def param(name, shape, dt, isOutput=False):
            return nc.dram_tensor(name, shape, dt)
    else:
        param = nc.declare_dram_parameter
    xT = param("xT", [128, EB * TL], BF16, isOutput=False)
    wq = param("wq", [NH * 128, EB * HD], BF16, isOutput=False)
    wo = param("wo", [(E // 512) * QB * 128, 512], BF16, isOutput=False)
    Ct = param("Ct", [HD, TL], F32, isOutput=False)
    St = param("St", [HD, TL], F32, isOutput=False)
    outT = param("outT", [E, TL], F32, isOutput=True)
    sink = (nc.declare_dram_parameter("sink", [128, 16], F32, isOutput=True)
            if timing else None)

    with TileContext(nc) as tc:
        loop_cm = (tc.For_i(0, loop_iters, 1) if loop_iters
                   else contextlib.nullcontext())
        with loop_cm:
            _emit_body(nc, tc, mybir, xT, wq, wo, Ct, St, outT)
        if timing:
            nc.sync.dma_start(out=sink[:], in_=outT[0:128, 0:16])

    nc.finalize()
    if TUNE["dedupe"]:
        _dedupe_ldweights(nc)
    return nc


def _dedupe_ldweights(nc):
    """Post-schedule BIR pass: drop an InstLdweights when its stationary
    access pattern is identical to the previous kept one, it carries no
    semaphores, and only wait-free InstMatmults sit between them (a waiting
    matmul could subsume a semaphore signalling a rewrite of the weights
    region).  Any other PE instruction resets the reference.  Safe because
    walrus MATMUL uses the persistently-loaded stationary operand."""
    def _sync_empty(inst):
        si = inst.sync_info
        return si is None or (not si.on_wait and not si.on_update)

    def _no_waits(inst):
        si = inst.sync_info
        return si is None or not si.on_wait

    for fn in nc.m.functions:
        for blk in fn.blocks:
            insts = blk.instructions
            ref_ap = None
            clean = True
            to_del = []
            for idx, inst in enumerate(insts):
                tname = type(inst).__name__
                if str(inst.engine) != "EngineType.PE":
                    continue
                if tname == "InstLdweights":
                    ap = str(inst.ins[0])
                    if ap == ref_ap and clean and _sync_empty(inst):
                        to_del.append(idx)
                    else:
                        ref_ap = ap
                        clean = True
                elif tname == "InstMatmult":
                    if not _no_waits(inst):
                        clean = False
                else:
                    ref_ap = None
                    clean = True
            for idx in reversed(to_del):
                del insts[idx]


def _emit_body(nc, tc, mybir, xT, wq, wo, Ct, St, outT):
    F32 = mybir.dt.float32
    BF16 = mybir.dt.bfloat16

    with tc.tile_pool(name="xp", bufs=1) as xp, \
         tc.tile_pool(name="qtp", bufs=1) as qtp:
        # persistent per-head RoPE'd q (bf16) — phase-2 moving operand
        qt = [qtp.tile([128, TL], BF16, tag=f"qt{h}", name=f"qt{h}")
              for h in range(NH)]

        # ---------------- phase 1: Q^T per head + RoPE ----------------------
        with tc.tile_pool(name="wqp", bufs=TUNE["wqp_bufs"]) as wqp, \
             tc.tile_pool(name="tab", bufs=1) as tab, \
             tc.tile_pool(name="rope", bufs=TUNE["rope_bufs"]) as rope, \
             tc.tile_pool(name="ps1", bufs=TUNE["ps1_bufs"],
                          space="PSUM") as ps1:
            nsp = TUNE["wq_split"]

            def load_wqh(h, nsplit):
                t = wqp.tile([128, EB * HD], BF16, tag="wqh")
                c = (EB * HD) // nsplit
                for i in range(nsplit):
                    nc.sync.dma_start(
                        out=t[:, i * c:(i + 1) * c],
                        in_=wq[h * 128:(h + 1) * 128, i * c:(i + 1) * c])
                return t

            # head-0 weights + X blocks gate the first matmuls — load first
            wqh0 = load_wqh(0, 8)   # fine-grained: first matmul starts early

            # X shard, one tile per e-block so matmuls only wait on their block
            xb = []
            for eb in range(EB):
                t = xp.tile([128, TL], BF16, tag=f"xb{eb}", name=f"xb{eb}")
                nc.sync.dma_start(out=t[:], in_=xT[:, eb * TL:(eb + 1) * TL])
                xb.append(t)

            # rope tables are only needed once head 0's PSUM is full (~13µs in)
            ct = tab.tile([HD, TL], F32, tag="ct")
            nc.sync.dma_start(out=ct[:], in_=Ct[:])
            stt = tab.tile([HD, TL], F32, tag="st")
            nc.sync.dma_start(out=stt[:], in_=St[:])

            for h in range(NH):
                wqh = wqh0 if h == 0 else load_wqh(h, nsp)
                pss = [ps1.tile([128, 512], F32, tag="ps1",
                                name=f"ps1_{h}_{i}") for i in range(2)]
                for eb in range(EB):
                    for tch in range(2):
                        nc.tensor.matmul(
                            pss[tch][:],
                            wqh[:, eb * HD:(eb + 1) * HD],
                            xb[eb][:, tch * 512:(tch + 1) * 512],
                            start=(eb == 0), stop=(eb == EB - 1),
                        )
                for tch in range(2):
                    ps = pss[tch]
                    sl = slice(tch * 512, tch * 512 + 512)
                    q0 = rope.tile([128, 512], F32, tag="q0")
                    u = rope.tile([128, 512], F32, tag="u")
                    qs = rope.tile([128, 512], F32, tag="qs")
                    v = rope.tile([128, 512], F32, tag="v")
                    # single PSUM read per group; rope math runs from SBUF
                    nc.vector.tensor_copy(q0[:], ps[:])
                    nc.vector.tensor_mul(u[:], q0[:], stt[:, sl])
                    nc.vector.tensor_mul(qs[:], q0[:], ct[:, sl])
                    # rotate-half: v = swap_halves(u) via partition-offset DMA
                    nc.sync.dma_start(out=v[0:HALF, :], in_=u[HALF:HD, :])
                    nc.sync.dma_start(out=v[HALF:HD, :], in_=u[0:HALF, :])
                    nc.vector.tensor_add(qt[h][:, sl], qs[:], v[:])

        # ---------------- phase 2: outT = sum_h wo[h].T @ qt[h] -------------
        with tc.tile_pool(name="wop", bufs=TUNE["wop_bufs"]) as wop, \
             tc.tile_pool(name="ost", bufs=TUNE["ost_bufs"]) as ost, \
             tc.tile_pool(name="ps2", bufs=8, space="PSUM") as ps2:
            for fq in range(E // 512):
                pss = [ps2.tile([128, 512], F32, tag="ps2",
                                name=f"pss_{fq}_{i}") for i in range(8)]
                for h in range(QB):
                    woh = wop.tile([128, 512], BF16, tag="woh")
                    r0 = (fq * QB + h) * 128
                    nc.sync.dma_start(out=woh[:], in_=wo[r0:r0 + 128, :])
                    for fb in range(4):
                        for t2 in range(2):
                            nc.tensor.matmul(
                                pss[fb * 2 + t2][:],
                                woh[:, fb * 128:(fb + 1) * 128],
                                qt[h][:, t2 * 512: t2 * 512 + 512],
                                start=(h == 0), stop=(h == QB - 1),
                            )
                for i in range(8):
                    fb, t2 = i // 2, i % 2
                    o = ost.tile([128, 512], F32, tag="ost")
                    nc.vector.tensor_copy(o[:], pss[i][:])
                    nc.sync.dma_start(
                        out=outT[fq * 512 + fb * 128: fq * 512 + (fb + 1) * 128,
                                 t2 * 512:(t2 + 1) * 512],
                        in_=o[:])


def _get_nc(loop_iters=None, timing=False):
    key = ("nc", loop_iters, timing)
    if key not in _NC_CACHE:
        _NC_CACHE[key] = _build_nc(loop_iters, timing)
    return _NC_CACHE[key]


def _rope_tables(positions):
    # mirrors reference fp32 math: inv_freq f32, freqs f32, cos/sin f32
    half = np.float32(HALF)
    inv_freq = (1.0 / (ROPE_THETA ** (np.arange(HALF, dtype=np.float32) / half))
                ).astype(np.float32)
    freqs = positions.astype(np.float32)[:, None] * inv_freq[None, :]  # [T, 64]
    cos = np.cos(freqs).astype(np.float32)
    sin = np.sin(freqs).astype(np.float32)
    # qT layout tables: Ct[d, t] = cos[t, d%64]
    # St[d, t] = +sin[t, d] for d<64, -sin[t, d-64] for d>=64, so that
    # qs + swap_halves(q * St) == neox rope of q.
    Ct = np.concatenate([cos.T, cos.T], axis=0)    # [128, T]
    St = np.concatenate([sin.T, -sin.T], axis=0)   # [128, T]
    return np.ascontiguousarray(Ct), np.ascontiguousarray(St)


def build_in_maps(hidden_states, positions, w_qkv, w_o):
    hidden = np.asarray(hidden_states, dtype=np.float32)
    pos = np.asarray(positions)
    wq_nat = np.asarray(w_qkv, dtype=np.float32)[:, :QS]
    wo_nat = np.asarray(w_o, dtype=np.float32)
    # pre-rearranged layouts (see _build_nc comments), cast to bf16 on host
    wq = np.ascontiguousarray(
        wq_nat.reshape(EB, 128, NH, HD).transpose(2, 1, 0, 3)
        .reshape(NH * 128, EB * HD).astype(BF16_NP))
    wo = np.ascontiguousarray(
        wo_nat.reshape(QB, 128, E // 512, 512).transpose(2, 0, 1, 3)
        .reshape((E // 512) * QB * 128, 512).astype(BF16_NP))
    Ct, St = _rope_tables(pos)
    in_maps = []
    for c in range(NCORES):
        sl = slice(c * TL, (c + 1) * TL)
        xTc = np.ascontiguousarray(
            hidden[sl].T.reshape(EB, 128, TL).transpose(1, 0, 2)
            .reshape(128, EB * TL).astype(BF16_NP))
        in_maps.append({
            "xT": xTc,
            "wq": wq,
            "wo": wo,
            "Ct": np.ascontiguousarray(Ct[:, sl]),
            "St": np.ascontiguousarray(St[:, sl]),
        })
    return in_maps


def kernel(hidden_states, positions, w_qkv, w_o):
    from concourse.bass_utils import run_bass_kernel_spmd

    nc = _get_nc()
    in_maps = build_in_maps(hidden_states, positions, w_qkv, w_o)
    res = run_bass_kernel_spmd(nc, in_maps, core_ids=list(range(NCORES)))
    out = np.concatenate(
        [np.asarray(res.results[c]["outT"]).T for c in range(NCORES)], axis=0)
    return np.ascontiguousarray(out.astype(np.float32))


# revision 5
# speedup vs baseline: 22423.9280x; 1.4750x over previous
"""Trainium2 Bass kernel for nn_CausalSelfAttention_22703197127379.

Reference computation (k/v are dead code — attention is stubbed to RoPE(q)):
    q    = hidden @ w_qkv[:, :4096]           # [8192, 4096]
    qr   = rope_neox(q, positions)            # per-head rotate-half RoPE
    out  = qr @ w_o                           # [8192, 4096]

Distribution: data-parallel over tokens — core c owns rows c*1024..(c+1)*1024.
No collectives; host concatenates the 8 shards.

Design (measured on HW: the TensorE stream saturates at ~1 bf16 row/cycle,
so everything else must hide under the 4096-matmul stream):
  * all matmul operands bf16 (PSUM accumulation stays f32): 3x less DMA +
    SBUF traffic than f32r; rel-err ~3e-3 vs the 2e-2 gate.
  * q stays resident in SBUF between the two matmuls — no qT DRAM bounce,
    no phase-2 reload stall.
  * X loaded as 32 per-e-block DMAs and weight tiles as split DMAs so the
    first matmuls start after ~100KB lands, not after the full 8MB.
  * RoPE reads each PSUM group once (copy to SBUF), math runs from SBUF;
    rotate-half via partition-swap DMA with signs folded into the host-built
    sin table.
  * post-schedule BIR pass drops LDWEIGHTS whose stationary matches the
    previous one (harmless; trims the instruction stream).

Per-core device kernel:
  phase 1: for each head h: Q^T[h] = sum_eb wq[eb,h].T @ x[eb] in PSUM;
           RoPE applied as qt = ps*C + swap_halves(ps*S), cast to bf16 into
           the persistent qt[h] SBUF tile.
  phase 2: outT[f,t] = sum_h wo[h,f].T @ qt[h], PSUM-accumulated over all
           32 head blocks, written transposed; host transposes back.
"""

import sys

if "/opt/trn_rl_repo" not in sys.path:
    sys.path.insert(0, "/opt/trn_rl_repo")

import numpy as np
import ml_dtypes

BF16_NP = ml_dtypes.bfloat16

NCORES = 8
T, E, QS = 8192, 4096, 4096
TL = T // NCORES          # 1024 tokens per core
NH = 32                   # q heads
HD = 128                  # head dim
HALF = HD // 2
EB = E // 128             # 32 contraction blocks
QB = QS // 128            # 32 head blocks
ROPE_THETA = 10000.0

_NC_CACHE = {}

TUNE = {
    "dedupe": True,
    "wq_split": 4,        # DMAs per wq head tile
    "ps1_bufs": 6,
    "wqp_bufs": 4,
    "wop_bufs": 16,
    "rope_bufs": 3,
    "ost_bufs": 8,
}


def _build_nc(loop_iters=None, timing=False):
    """Build the per-core NEFF. loop_iters wraps the compute body in a
    hardware For_i loop (timing-only builds; data goes stale after iter 0).
    timing=True swaps all I/O parameters for internal DRAM tensors (garbage
    contents, nothing shipped over the tunnel) plus a tiny sink output."""
    import contextlib

    import concourse.bacc as bacc
    import concourse.mybir as mybir
    from concourse.tile import TileContext

    F32 = mybir.dt.float32
    BF16 = mybir.dt.bfloat16

    nc = bacc.Bacc()
    # all inputs arrive pre-rearranged on host so every DMA is contiguous:
    # xT[p, eb*TL + t]            = bf16(hidden_shard.T)[eb*128 + p, t]
    # wq[h*128 + p, eb*HD + f]    = bf16(w_q)[eb*128 + p, h*HD + f]
    # wo[(fq*QB + h)*128 + p, f]  = bf16(w_o)[h*128 + p, fq*512 + f]
    if timing:
        def param(name, shape, dt, isOutput=False):
            return nc.dram_tensor(name, shape, dt)
    else:
        param = nc.declare_dram_parameter
    xT = param("xT", [128, EB * TL], BF16, isOutput=False)
    wq = param("wq", [NH * 128, EB * HD], BF16, isOutput=False)
    wo = param("wo", [(E // 512) * QB * 128, 512], BF16, isOutput=False)
    Ct = param("Ct", [HD, TL], F32, isOutput=False)
    St = param("St", [HD, TL], F32, isOutput=False)
    outT = param("outT", [E, TL], F32, isOutput=True)
    sink = (nc.declare_dram_parameter("sink", [128, 16], F32, isOutput=True)
            if timing else None)

    with TileContext(nc) as tc:
        loop_cm = (tc.For_i(0, loop_iters, 1) if loop_iters
                   else contextlib.nullcontext())
        with loop_cm:
            _emit_body(nc, tc, mybir, xT, wq, wo, Ct, St, outT)
        if timing:
            nc.sync.dma_start(out=sink[:], in_=outT[0:128, 0:16])

    nc.finalize()
    if TUNE["dedupe"]:
        _dedupe_ldweights(nc)
    return nc


def _dedupe_ldweights(nc):
    """Post-schedule BIR pass: drop an InstLdweights when its stationary
    access pattern is identical to the previous kept one, it carries no
    semaphores, and only wait-free InstMatmults sit between them (a waiting
    matmul could subsume a semaphore signalling a rewrite of the weights
    region).  Any other PE instruction resets the reference.  Safe because
    walrus MATMUL uses the persistently-loaded stationary operand."""
    def _sync_empty(inst):
        si = inst.sync_info
        return si is None or (not si.on_wait and not si.on_update)

    def _no_waits(inst):
        si = inst.sync_info
        return si is None or not si.on_wait

    for fn in nc.m.functions:
        for blk in fn.blocks:
            insts = blk.instructions
            ref_ap = None
            clean = True
            to_del = []
            for idx, inst in enumerate(insts):
                tname = type(inst).__name__
                if str(inst.engine) != "EngineType.PE":
                    continue
                if tname == "InstLdweights":
                    ap = str(inst.ins[0])
                    if ap == ref_ap and clean and _sync_empty(inst):
                        to_del.append(idx)
                    else:
                        ref_ap = ap
                        clean = True
                elif tname == "InstMatmult":
                    if not _no_waits(inst):
                        clean = False
                else:
                    ref_ap = None
                    clean = True
            for idx in reversed(to_del):
                del insts[idx]


def _emit_body(nc, tc, mybir, xT, wq, wo, Ct, St, outT):
    F32 = mybir.dt.float32
    BF16 = mybir.dt.bfloat16

    with tc.tile_pool(name="xp", bufs=1) as xp, \
         tc.tile_pool(name="qtp", bufs=1) as qtp:
        # persistent per-head RoPE'd q (bf16) — phase-2 moving operand
        qt = [qtp.tile([128, TL], BF16, tag=f"qt{h}", name=f"qt{h}")
              for h in range(NH)]

        # ---------------- phase 1: Q^T per head + RoPE ----------------------
        with tc.tile_pool(name="wqp", bufs=TUNE["wqp_bufs"]) as wqp, \
             tc.tile_pool(name="tab", bufs=1) as tab, \
             tc.tile_pool(name="rope", bufs=TUNE["rope_bufs"]) as rope, \
             tc.tile_pool(name="ps1", bufs=TUNE["ps1_bufs"],
                          space="PSUM") as ps1:
            nsp = TUNE["wq_split"]

            def load_wqh(h, nsplit):
                t = wqp.tile([128, EB * HD], BF16, tag="wqh")
                c = (EB * HD) // nsplit
                for i in range(nsplit):
                    nc.sync.dma_start(
                        out=t[:, i * c:(i + 1) * c],
                        in_=wq[h * 128:(h + 1) * 128, i * c:(i + 1) * c])
                return t

            # head-0 weights + X blocks gate the first matmuls — load first
            wqh0 = load_wqh(0, 8)   # fine-grained: first matmul starts early

            # X shard, one tile per e-block so matmuls only wait on their block
            xb = []
            for eb in range(EB):
                t = xp.tile([128, TL], BF16, tag=f"xb{eb}", name=f"xb{eb}")
                nc.sync.dma_start(out=t[:], in_=xT[:, eb * TL:(eb + 1) * TL])
                xb.append(t)

            # rope tables are only needed once head 0's PSUM is full (~13µs in)
            ct = tab.tile([HD, TL], F32, tag="ct")
            nc.sync.dma_start(out=ct[:], in_=Ct[:])
            stt = tab.tile([HD, TL], F32, tag="st")
            nc.sync.dma_start(out=stt[:], in_=St[:])

            for h in range(NH):
                wqh = wqh0 if h == 0 else load_wqh(h, nsp)
                pss = [ps1.tile([128, 512], F32, tag="ps1",
                                name=f"ps1_{h}_{i}") for i in range(2)]
                for eb in range(EB):
                    for tch in range(2):
                        nc.tensor.matmul(
                            pss[tch][:],
                            wqh[:, eb * HD:(eb + 1) * HD],
                            xb[eb][:, tch * 512:(tch + 1) * 512],
                            start=(eb == 0), stop=(eb == EB - 1),
                        )
                for tch in range(2):
                    ps = pss[tch]
                    sl = slice(tch * 512, tch * 512 + 512)
                    q0 = rope.tile([128, 512], F32, tag="q0")
                    u = rope.tile([128, 512], F32, tag="u")
                    qs = rope.tile([128, 512], F32, tag="qs")
                    v = rope.tile([128, 512], F32, tag="v")
                    # single PSUM read per group; rope math runs from SBUF
                    nc.vector.tensor_copy(q0[:], ps[:])
                    nc.vector.tensor_mul(u[:], q0[:], stt[:, sl])
                    nc.vector.tensor_mul(qs[:], q0[:], ct[:, sl])
                    # rotate-half: v = swap_halves(u) via partition-offset DMA
                    nc.sync.dma_start(out=v[0:HALF, :], in_=u[HALF:HD, :])
                    nc.sync.dma_start(out=v[HALF:HD, :], in_=u[0:HALF, :])
                    nc.vector.tensor_add(qt[h][:, sl], qs[:], v[:])

        # ---------------- phase 2: outT = sum_h wo[h].T @ qt[h] -------------
        with tc.tile_pool(name="wop", bufs=TUNE["wop_bufs"]) as wop, \
             tc.tile_pool(name="ost", bufs=TUNE["ost_bufs"]) as ost, \
             tc.tile_pool(name="ps2", bufs=8, space="PSUM") as ps2:
            for fq in range(E // 512):
                pss = [ps2.tile([128, 512], F32, tag="ps2",
                                name=f"pss_{fq}_{i}") for i in range(8)]
                for h in range(QB):
                    woh = wop.tile([128, 512], BF16, tag="woh")
                    r0 = (fq * QB + h) * 128
                    nc.sync.dma_start(out=woh[:], in_=wo[r0:r0 + 128, :])
                    for fb in range(4):
                        for t2 in range(2):
                            nc.tensor.matmul(
                                pss[fb * 2 + t2][:],
                                woh[:, fb * 128:(fb + 1) * 128],
                                qt[h][:, t2 * 512: t2 * 512 + 512],
                                start=(h == 0), stop=(h == QB - 1),
                            )
                for i in range(8):
                    fb, t2 = i // 2, i % 2
                    o = ost.tile([128, 512], F32, tag="ost")
                    nc.vector.tensor_copy(o[:], pss[i][:])
                    nc.sync.dma_start(
                        out=outT[fq * 512 + fb * 128: fq * 512 + (fb + 1) * 128,
                                 t2 * 512:(t2 + 1) * 512],
                        in_=o[:])


def _get_nc(loop_iters=None, timing=False):
    key = ("nc", loop_iters, timing)
    if key not in _NC_CACHE:
        _NC_CACHE[key] = _build_nc(loop_iters, timing)
    return _NC_CACHE[key]


def _rope_tables(positions):
    # mirrors reference fp32 math: inv_freq f32, freqs f32, cos/sin f32
    half = np.float32(HALF)
    inv_freq = (1.0 / (ROPE_THETA ** (np.arange(HALF, dtype=np.float32) / half))
                ).astype(np.float32)
    freqs = positions.astype(np.float32)[:, None] * inv_freq[None, :]  # [T, 64]
    cos = np.cos(freqs).astype(np.float32)
    sin = np.sin(freqs).astype(np.float32)
    # qT layout tables: Ct[d, t] = cos[t, d%64]
    # St[d, t] = +sin[t, d] for d<64, -sin[t, d-64] for d>=64, so that
    # qs + swap_halves(q * St) == neox rope of q.
    Ct = np.concatenate([cos.T, cos.T], axis=0)    # [128, T]
    St = np.concatenate([sin.T, -sin.T], axis=0)   # [128, T]
    return np.ascontiguousarray(Ct), np.ascontiguousarray(St)


def build_in_maps(hidden_states, positions, w_qkv, w_o):
    hidden = np.asarray(hidden_states, dtype=np.float32)
    pos = np.asarray(positions)
    wq_nat = np.asarray(w_qkv, dtype=np.float32)[:, :QS]
    wo_nat = np.asarray(w_o, dtype=np.float32)
    # pre-rearranged layouts (see _build_nc comments), cast to bf16 on host
    wq = np.ascontiguousarray(
        wq_nat.reshape(EB, 128, NH, HD).transpose(2, 1, 0, 3)
        .reshape(NH * 128, EB * HD).astype(BF16_NP))
    wo = np.ascontiguousarray(
        wo_nat.reshape(QB, 128, E // 512, 512).transpose(2, 0, 1, 3)
        .reshape((E // 512) * QB * 128, 512).astype(BF16_NP))
    Ct, St = _rope_tables(pos)
    in_maps = []
    for c in range(NCORES):
        sl = slice(c * TL, (c + 1) * TL)
        xTc = np.ascontiguousarray(
            hidden[sl].T.reshape(EB, 128, TL).transpose(1, 0, 2)
            .reshape(128, EB * TL).astype(BF16_NP))
        in_maps.append({
            "xT": xTc,
            "wq": wq,
            "wo": wo,
            "Ct": np.ascontiguousarray(Ct[:, sl]),
            "St": np.ascontiguousarray(St[:, sl]),
        })
    return in_maps


def kernel(hidden_states, positions, w_qkv, w_o):
    from concourse.bass_utils import run_bass_kernel_spmd

    nc = _get_nc()
    in_maps = build_in_maps(hidden_states, positions, w_qkv, w_o)
    res = run_bass_kernel_spmd(nc, in_maps, core_ids=list(range(NCORES)))
    out = np.concatenate(
        [np.asarray(res.results[c]["outT"]).T for c in range(NCORES)], axis=0)
    return np.ascontiguousarray(out.astype(np.float32))


# revision 8
# speedup vs baseline: 25214.6788x; 1.1245x over previous
"""Trainium2 Bass kernel for nn_CausalSelfAttention_22703197127379.

Reference computation (k/v are dead code — attention is stubbed to RoPE(q)):
    q    = hidden @ w_qkv[:, :4096]           # [8192, 4096]
    qr   = rope_neox(q, positions)            # per-head rotate-half RoPE
    out  = qr @ w_o                           # [8192, 4096]

Distribution: data-parallel over tokens — core c owns rows c*1024..(c+1)*1024.
No collectives; host concatenates the 8 shards.

Design (measured on HW: the TensorE stream saturates at ~1 bf16 row/cycle,
so everything else must hide under the 4096-matmul stream):
  * all matmul operands bf16 (PSUM accumulation stays f32): 3x less DMA +
    SBUF traffic than f32r; rel-err ~3e-3 vs the 2e-2 gate.
  * q stays resident in SBUF between the two matmuls — no qT DRAM bounce,
    no phase-2 reload stall.
  * X loaded as 32 per-e-block DMAs and weight tiles as split DMAs so the
    first matmuls start after ~100KB lands, not after the full 8MB.
  * RoPE reads each PSUM group once (copy to SBUF), math runs from SBUF;
    rotate-half via partition-swap DMA with signs folded into the host-built
    sin table.
  * post-schedule BIR pass drops LDWEIGHTS whose stationary matches the
    previous one (harmless; trims the instruction stream).

Per-core device kernel:
  phase 1: for each head h: Q^T[h] = sum_eb wq[eb,h].T @ x[eb] in PSUM;
           RoPE applied as qt = ps*C + swap_halves(ps*S), cast to bf16 into
           the persistent qt[h] SBUF tile.
  phase 2: outT[f,t] = sum_h wo[h,f].T @ qt[h], PSUM-accumulated over all
           32 head blocks, written transposed; host transposes back.
"""

import sys

if "/opt/trn_rl_repo" not in sys.path:
    sys.path.insert(0, "/opt/trn_rl_repo")

import numpy as np
import ml_dtypes

BF16_NP = ml_dtypes.bfloat16

NCORES = 8
T, E, QS = 8192, 4096, 4096
TL = T // NCORES          # 1024 tokens per core
NH = 32                   # q heads
HD = 128                  # head dim
HALF = HD // 2
EB = E // 128             # 32 contraction blocks
QB = QS // 128            # 32 head blocks
ROPE_THETA = 10000.0

_NC_CACHE = {}

TUNE = {
    "dedupe": True,
    "wq_split": 4,        # DMAs per wq head tile
    "ps1_bufs": 6,
    "wqp_bufs": 4,
    "wop_bufs": 16,
    "rope_bufs": 3,
    "ost_bufs": 8,
    "ph2_order": 0,       # 0: h outer (stream wo); 1: chain per output tile
}


def _build_nc(loop_iters=None, timing=False):
    """Build the per-core NEFF. loop_iters wraps the compute body in a
    hardware For_i loop (timing-only builds; data goes stale after iter 0).
    timing=True swaps all I/O parameters for internal DRAM tensors (garbage
    contents, nothing shipped over the tunnel) plus a tiny sink output."""
    import contextlib

    import concourse.bacc as bacc
    import concourse.mybir as mybir
    from concourse.tile import TileContext

    F32 = mybir.dt.float32
    BF16 = mybir.dt.bfloat16

    nc = bacc.Bacc()
    # all inputs arrive pre-rearranged on host so every DMA is contiguous:
    # xT[p, eb*TL + t]            = bf16(hidden_shard.T)[eb*128 + p, t]
    # wq[h*128 + p, eb*HD + f]    = bf16(w_q)[eb*128 + p, h*HD + f]
    # wo[(fq*QB + h)*128 + p, f]  = bf16(w_o)[h*128 + p, fq*512 + f]
    if timing:
        def param(name, shape, dt, isOutput=False):
            return nc.dram_tensor(name, shape, dt)
    else:
        param = nc.declare_dram_parameter
    xT = param("xT", [128, EB * TL], BF16, isOutput=False)
    wq = param("wq", [NH * 128, EB * HD], BF16, isOutput=False)
    wo = param("wo", [(E // 512) * QB * 128, 512], BF16, isOutput=False)
    Ct = param("Ct", [HD, TL], F32, isOutput=False)
    St = param("St", [HD, TL], F32, isOutput=False)
    outT = param("outT", [E, TL], F32, isOutput=True)
    sink = (nc.declare_dram_parameter("sink", [128, 16], F32, isOutput=True)
            if timing else None)

    with TileContext(nc) as tc:
        loop_cm = (tc.For_i(0, loop_iters, 1) if loop_iters
                   else contextlib.nullcontext())
        with loop_cm:
            _emit_body(nc, tc, mybir, xT, wq, wo, Ct, St, outT)
        if timing:
            nc.sync.dma_start(out=sink[:], in_=outT[0:128, 0:16])

    nc.finalize()
    if TUNE["dedupe"]:
        _dedupe_ldweights(nc)
    return nc


def _dedupe_ldweights(nc):
    """Post-schedule BIR pass: drop an InstLdweights when its stationary
    access pattern is identical to the previous kept one, it carries no
    semaphores, and only wait-free InstMatmults sit between them (a waiting
    matmul could subsume a semaphore signalling a rewrite of the weights
    region).  Any other PE instruction resets the reference.  Safe because
    walrus MATMUL uses the persistently-loaded stationary operand."""
    def _sync_empty(inst):
        si = inst.sync_info
        return si is None or (not si.on_wait and not si.on_update)

    def _no_waits(inst):
        si = inst.sync_info
        return si is None or not si.on_wait

    for fn in nc.m.functions:
        for blk in fn.blocks:
            insts = blk.instructions
            ref_ap = None
            clean = True
            to_del = []
            for idx, inst in enumerate(insts):
                tname = type(inst).__name__
                if str(inst.engine) != "EngineType.PE":
                    continue
                if tname == "InstLdweights":
                    ap = str(inst.ins[0])
                    if ap == ref_ap and clean and _sync_empty(inst):
                        to_del.append(idx)
                    else:
                        ref_ap = ap
                        clean = True
                elif tname == "InstMatmult":
                    if not _no_waits(inst):
                        clean = False
                else:
                    ref_ap = None
                    clean = True
            for idx in reversed(to_del):
                del insts[idx]


def _emit_body(nc, tc, mybir, xT, wq, wo, Ct, St, outT):
    F32 = mybir.dt.float32
    BF16 = mybir.dt.bfloat16

    with tc.tile_pool(name="xp", bufs=1) as xp, \
         tc.tile_pool(name="qtp", bufs=1) as qtp:
        # persistent per-head RoPE'd q (bf16) — phase-2 moving operand
        qt = [qtp.tile([128, TL], BF16, tag=f"qt{h}", name=f"qt{h}")
              for h in range(NH)]

        # ---------------- phase 1: Q^T per head + RoPE ----------------------
        with tc.tile_pool(name="wqp", bufs=TUNE["wqp_bufs"]) as wqp, \
             tc.tile_pool(name="tab", bufs=1) as tab, \
             tc.tile_pool(name="rope", bufs=TUNE["rope_bufs"]) as rope, \
             tc.tile_pool(name="ps1", bufs=TUNE["ps1_bufs"],
                          space="PSUM") as ps1:
            nsp = TUNE["wq_split"]

            def load_wqh(h, nsplit):
                t = wqp.tile([128, EB * HD], BF16, tag="wqh")
                c = (EB * HD) // nsplit
                for i in range(nsplit):
                    nc.sync.dma_start(
                        out=t[:, i * c:(i + 1) * c],
                        in_=wq[h * 128:(h + 1) * 128, i * c:(i + 1) * c])
                return t

            # head-0 weights + X blocks gate the first matmuls — load first
            wqh0 = load_wqh(0, 8)   # fine-grained: first matmul starts early

            # X shard, one tile per e-block so matmuls only wait on their block
            xb = []
            for eb in range(EB):
                t = xp.tile([128, TL], BF16, tag=f"xb{eb}", name=f"xb{eb}")
                nc.sync.dma_start(out=t[:], in_=xT[:, eb * TL:(eb + 1) * TL])
                xb.append(t)

            # rope tables are only needed once head 0's PSUM is full (~13µs in)
            ct = tab.tile([HD, TL], F32, tag="ct")
            nc.sync.dma_start(out=ct[:], in_=Ct[:])
            stt = tab.tile([HD, TL], F32, tag="st")
            nc.sync.dma_start(out=stt[:], in_=St[:])

            for h in range(NH):
                wqh = wqh0 if h == 0 else load_wqh(h, nsp)
                pss = [ps1.tile([128, 512], F32, tag="ps1",
                                name=f"ps1_{h}_{i}") for i in range(2)]
                for eb in range(EB):
                    for tch in range(2):
                        nc.tensor.matmul(
                            pss[tch][:],
                            wqh[:, eb * HD:(eb + 1) * HD],
                            xb[eb][:, tch * 512:(tch + 1) * 512],
                            start=(eb == 0), stop=(eb == EB - 1),
                        )
                for tch in range(2):
                    ps = pss[tch]
                    sl = slice(tch * 512, tch * 512 + 512)
                    q0 = rope.tile([128, 512], F32, tag="q0")
                    u = rope.tile([128, 512], F32, tag="u")
                    qs = rope.tile([128, 512], F32, tag="qs")
                    v = rope.tile([128, 512], F32, tag="v")
                    # single PSUM read per group; rope math runs from SBUF
                    nc.vector.tensor_copy(q0[:], ps[:])
                    nc.vector.tensor_mul(u[:], q0[:], stt[:, sl])
                    nc.vector.tensor_mul(qs[:], q0[:], ct[:, sl])
                    # rotate-half: v = swap_halves(u) via partition-offset DMA
                    nc.sync.dma_start(out=v[0:HALF, :], in_=u[HALF:HD, :])
                    nc.sync.dma_start(out=v[HALF:HD, :], in_=u[0:HALF, :])
                    nc.vector.tensor_add(qt[h][:, sl], qs[:], v[:])

        # ---------------- phase 2: outT = sum_h wo[h].T @ qt[h] -------------
        wop_bufs = max(TUNE["wop_bufs"], 48) if TUNE["ph2_order"] \
            else TUNE["wop_bufs"]
        with tc.tile_pool(name="wop", bufs=wop_bufs) as wop, \
             tc.tile_pool(name="ost", bufs=TUNE["ost_bufs"]) as ost, \
             tc.tile_pool(name="ps2", bufs=8, space="PSUM") as ps2:
            if TUNE["ph2_order"]:
                # chain per output tile: each PSUM bank runs its full 32-MM
                # contraction then drains immediately — drains spread across
                # the whole span and banks free early for the next fq.
                for fq in range(E // 512):
                    wohs = []
                    for h in range(QB):
                        woh = wop.tile([128, 512], BF16, tag="woh",
                                       name=f"woh_{fq}_{h}")
                        r0 = (fq * QB + h) * 128
                        nc.sync.dma_start(out=woh[:], in_=wo[r0:r0 + 128, :])
                        wohs.append(woh)
                    for fb in range(4):
                        for t2 in range(2):
                            ps = ps2.tile([128, 512], F32, tag="ps2",
                                          name=f"pss_{fq}_{fb}_{t2}")
                            for h in range(QB):
                                nc.tensor.matmul(
                                    ps[:],
                                    wohs[h][:, fb * 128:(fb + 1) * 128],
                                    qt[h][:, t2 * 512: t2 * 512 + 512],
                                    start=(h == 0), stop=(h == QB - 1),
                                )
                            o = ost.tile([128, 512], F32, tag="ost")
                            nc.vector.tensor_copy(o[:], ps[:])
                            nc.sync.dma_start(
                                out=outT[fq * 512 + fb * 128:
                                         fq * 512 + (fb + 1) * 128,
                                         t2 * 512:(t2 + 1) * 512],
                                in_=o[:])
                return

            for fq in range(E // 512):
                pss = [ps2.tile([128, 512], F32, tag="ps2",
                                name=f"pss_{fq}_{i}") for i in range(8)]
                for h in range(QB):
                    woh = wop.tile([128, 512], BF16, tag="woh")
                    r0 = (fq * QB + h) * 128
                    nc.sync.dma_start(out=woh[:], in_=wo[r0:r0 + 128, :])
                    for fb in range(4):
                        for t2 in range(2):
                            nc.tensor.matmul(
                                pss[fb * 2 + t2][:],
                                woh[:, fb * 128:(fb + 1) * 128],
                                qt[h][:, t2 * 512: t2 * 512 + 512],
                                start=(h == 0), stop=(h == QB - 1),
                            )
                for i in range(8):
                    fb, t2 = i // 2, i % 2
                    o = ost.tile([128, 512], F32, tag="ost")
                    nc.vector.tensor_copy(o[:], pss[i][:])
                    nc.sync.dma_start(
                        out=outT[fq * 512 + fb * 128: fq * 512 + (fb + 1) * 128,
                                 t2 * 512:(t2 + 1) * 512],
                        in_=o[:])


def _get_nc(loop_iters=None, timing=False):
    key = ("nc", loop_iters, timing)
    if key not in _NC_CACHE:
        _NC_CACHE[key] = _build_nc(loop_iters, timing)
    return _NC_CACHE[key]


def _rope_tables(positions):
    # mirrors reference fp32 math: inv_freq f32, freqs f32, cos/sin f32
    half = np.float32(HALF)
    inv_freq = (1.0 / (ROPE_THETA ** (np.arange(HALF, dtype=np.float32) / half))
                ).astype(np.float32)
    freqs = positions.astype(np.float32)[:, None] * inv_freq[None, :]  # [T, 64]
    cos = np.cos(freqs).astype(np.float32)
    sin = np.sin(freqs).astype(np.float32)
    # qT layout tables: Ct[d, t] = cos[t, d%64]
    # St[d, t] = +sin[t, d] for d<64, -sin[t, d-64] for d>=64, so that
    # qs + swap_halves(q * St) == neox rope of q.
    Ct = np.concatenate([cos.T, cos.T], axis=0)    # [128, T]
    St = np.concatenate([sin.T, -sin.T], axis=0)   # [128, T]
    return np.ascontiguousarray(Ct), np.ascontiguousarray(St)


def build_in_maps(hidden_states, positions, w_qkv, w_o):
    hidden = np.asarray(hidden_states, dtype=np.float32)
    pos = np.asarray(positions)
    wq_nat = np.asarray(w_qkv, dtype=np.float32)[:, :QS]
    wo_nat = np.asarray(w_o, dtype=np.float32)
    # pre-rearranged layouts (see _build_nc comments), cast to bf16 on host
    wq = np.ascontiguousarray(
        wq_nat.reshape(EB, 128, NH, HD).transpose(2, 1, 0, 3)
        .reshape(NH * 128, EB * HD).astype(BF16_NP))
    wo = np.ascontiguousarray(
        wo_nat.reshape(QB, 128, E // 512, 512).transpose(2, 0, 1, 3)
        .reshape((E // 512) * QB * 128, 512).astype(BF16_NP))
    Ct, St = _rope_tables(pos)
    in_maps = []
    for c in range(NCORES):
        sl = slice(c * TL, (c + 1) * TL)
        xTc = np.ascontiguousarray(
            hidden[sl].T.reshape(EB, 128, TL).transpose(1, 0, 2)
            .reshape(128, EB * TL).astype(BF16_NP))
        in_maps.append({
            "xT": xTc,
            "wq": wq,
            "wo": wo,
            "Ct": np.ascontiguousarray(Ct[:, sl]),
            "St": np.ascontiguousarray(St[:, sl]),
        })
    return in_maps


def kernel(hidden_states, positions, w_qkv, w_o):
    from concourse.bass_utils import run_bass_kernel_spmd

    nc = _get_nc()
    in_maps = build_in_maps(hidden_states, positions, w_qkv, w_o)
    res = run_bass_kernel_spmd(nc, in_maps, core_ids=list(range(NCORES)))
    out = np.concatenate(
        [np.asarray(res.results[c]["outT"]).T for c in range(NCORES)], axis=0)
    return np.ascontiguousarray(out.astype(np.float32))
